# revision 1
# baseline (speedup 1.0000x reference)
"""Two-layer GRU encoder (B=64, T=12, N=325, D=2, H=256) on 8 TRN2 NeuronCores.

Strategy: data-parallel over batch (8 B-slices, one per core; per-core row
count M = 8*325 = 2600). Everything on-device uses a transposed
"feature-on-partition" layout: hidden state h is stored as (128, 2*m) bf16
tiles whose halves are feature chunks [0:128] and [128:256]; GRU weights sit
stationary in the PE as bf16 lhsT tiles and the batch dimension streams as
the matmul moving operand in chunks of 434/432 rows.

Per step/chunk/layer: the x-projection matmul (K=3 for layer 0 — x0, x1
plus an all-ones row that carries the combined biases; K=256 from h0' for
layer 1) accumulates with the recurrent matmul (K=256) directly in PSUM
(one bank per gate-feature chunk, batch chunk <= 512); sigmoid/tanh run on
the scalar engine (layer-1 biases via the per-partition bias operand); r*h
and the state blend h' = h + z*(c - h) run on the vector engine in bf16
(2x mode). Emission is software-pipelined (candidate stage one chunk behind
the z/r stage; all layer-0 chunks before layer-1 per step) so the in-order
PE instruction stream never waits on the sigmoid -> r*h chain it just fed.

The host wrapper shards/transposes inputs, runs the SPMD kernel via
run_bass_kernel_spmd on cores 0-7, and reassembles the (2, 64, 325, 256)
float32 output.
"""

import numpy as np
import ml_dtypes
from contextlib import ExitStack

import concourse.bass as bass
import concourse.tile as tile
from concourse import bacc, mybir
from concourse import bass_utils

BF16 = ml_dtypes.bfloat16
AF = mybir.ActivationFunctionType

H = 256
T = 12
B = 64
N = 325
D = 2
NCORES = 8
B_SH = B // NCORES            # 8
M = B_SH * N                  # 2600
_CWS = [434, 434, 434, 434, 432, 432]   # even, <=512 (PSUM bank), sum = 2600
CHUNKS = []
_o = 0
for _w in _CWS:
    CHUNKS.append((_o, _w))
    _o += _w
CWMAX = max(_CWS)
DT = mybir.dt

_CACHE = {}


def _emit_zr_stage(nc, psum, work, t, mw, emit_xp, whzr_sb, bias_sb, bcol0,
                   h_prev, uid):
    """Stage A of one GRU cell: z/r pre-activations (PSUM), sigmoids, r*h.

    emit_xp(g, out_ap, more): emits the x-projection matmuls into psum slice
    `out_ap` for gate-feature chunk g (0..5 = za,zb,ra,rb,ca,cb); `more` is
    True when recurrent matmuls will accumulate on top afterwards.
    gate order in weight cols: z:[0:256] r:[256:512] c:[512:768].
    bcol0 is None when the layer's biases already rode the x-projection
    matmul (ones-row trick) — the sigmoids then fuse both feature halves.
    Returns (s_z, rh) for stage B.
    """
    first = t == 0
    f32 = DT.float32

    ptiles = {}
    for gi in (2, 0):  # r first: it's on the critical path
        for half in (0, 1):
            g = gi + half
            pt = psum.tile([128, mw], f32, tag="ps", name=f"p{uid}_g{g}")
            emit_xp(g, pt[:], more=not first)
            if not first:
                for k in (0, 1):
                    nc.tensor.matmul(
                        pt[:],
                        whzr_sb[:, k * 512 + g * 128: k * 512 + (g + 1) * 128],
                        h_prev[:, k * mw:(k + 1) * mw],
                        start=False, stop=(k == 1),
                    )
            ptiles[g] = pt

    s_r = work.tile([128, 2 * mw], DT.bfloat16, tag="sr", name=f"sr{uid}")
    s_z = work.tile([128, 2 * mw], DT.bfloat16, tag="sz", name=f"sz{uid}")
    for half in (0, 1):
        kw = {} if bcol0 is None else dict(
            bias=bias_sb[:, bcol0 + 2 + half: bcol0 + 3 + half])
        nc.scalar.activation(s_r[:, half * mw:(half + 1) * mw],
                             ptiles[2 + half][:], AF.Sigmoid, **kw)
    for half in (0, 1):
        kw = {} if bcol0 is None else dict(
            bias=bias_sb[:, bcol0 + half: bcol0 + 1 + half])
        nc.scalar.activation(s_z[:, half * mw:(half + 1) * mw],
                             ptiles[half][:], AF.Sigmoid, **kw)

    rh = None
    if not first:
        rh = work.tile([128, 2 * mw], DT.bfloat16, tag="rh", name=f"rh{uid}")
        nc.vector.tensor_mul(rh[:], s_r[:], h_prev[:])
    return s_z, rh


def _emit_cand_stage(nc, psum, work, t, mw, emit_xp, whh_sb, bias_sb, bcol0,
                     h_prev, h_new, s_z, rh, uid):
    """Stage B: candidate matmuls + tanh + state blend h' = h + z*(c-h)."""
    first = t == 0
    f32 = DT.float32

    pcs = []
    for half in (0, 1):
        g = 4 + half
        pt = psum.tile([128, mw], f32, tag="ps", name=f"p{uid}_g{g}")
        emit_xp(g, pt[:], more=not first)
        if not first:
            for k in (0, 1):
                nc.tensor.matmul(
                    pt[:],
                    whh_sb[:, k * 256 + half * 128: k * 256 + (half + 1) * 128],
                    rh[:, k * mw:(k + 1) * mw],
                    start=False, stop=(k == 1),
                )
        pcs.append(pt)

    c = work.tile([128, 2 * mw], DT.bfloat16, tag="c", name=f"c{uid}")
    for half in (0, 1):
        kw = {} if bcol0 is None else dict(
            bias=bias_sb[:, bcol0 + 4 + half: bcol0 + 5 + half])
        nc.scalar.activation(c[:, half * mw:(half + 1) * mw],
                             pcs[half][:], AF.Tanh, **kw)

    if first:
        nc.vector.tensor_mul(h_new[:], s_z[:], c[:])
    else:
        d = work.tile([128, 2 * mw], DT.bfloat16, tag="d", name=f"d{uid}")
        nc.vector.tensor_sub(d[:], c[:], h_prev[:])
        zd = work.tile([128, 2 * mw], DT.bfloat16, tag="zd", name=f"zd{uid}")
        nc.vector.tensor_mul(zd[:], s_z[:], d[:])
        nc.vector.tensor_add(h_new[:], h_prev[:], zd[:])


def _build_nc():
    nc = bacc.Bacc("TRN2", target_bir_lowering=False, debug=False,
                   enable_asserts=False)
    bf = DT.bfloat16

    xt_d = nc.dram_tensor("xt", (D + 1, T * M), bf, kind="ExternalInput").ap()
    wx0_d = nc.dram_tensor("wx0", (D + 1, 768), bf, kind="ExternalInput").ap()
    whzr0_d = nc.dram_tensor("whzr0", (128, 1024), bf, kind="ExternalInput").ap()
    whh0_d = nc.dram_tensor("whh0", (128, 512), bf, kind="ExternalInput").ap()
    wx1_d = nc.dram_tensor("wx1", (128, 1536), bf, kind="ExternalInput").ap()
    whzr1_d = nc.dram_tensor("whzr1", (128, 1024), bf, kind="ExternalInput").ap()
    whh1_d = nc.dram_tensor("whh1", (128, 512), bf, kind="ExternalInput").ap()
    bias_d = nc.dram_tensor("bias", (128, 12), DT.float32, kind="ExternalInput").ap()
    out_d = nc.dram_tensor("out", (2, len(CHUNKS), 128, 2 * CWMAX), bf,
                           kind="ExternalOutput").ap()

    with tile.TileContext(nc) as tc, ExitStack() as ctx:
        const = ctx.enter_context(tc.tile_pool(name="const", bufs=1))
        hpool = ctx.enter_context(tc.tile_pool(name="hstate", bufs=1))
        work = ctx.enter_context(tc.tile_pool(name="work", bufs=5))
        psum = ctx.enter_context(tc.tile_pool(name="psum", bufs=8, space="PSUM"))

        def load(name, dram, shape, dtype=bf):
            t_ = const.tile(list(shape), dtype, tag=name, name=name)
            nc.sync.dma_start(t_[:], dram[:])
            return t_

        wx0_sb = load("wx0", wx0_d, (D + 1, 768))
        # xt uses only 3 SBUF partitions, so one monolithic DMA is port-starved
        # (~13us) and gates the first matmul; split per timestep so t=0 compute
        # starts after a 1/12-sized slice and the rest streams behind compute.
        xt_sb = const.tile([D + 1, T * M], bf, tag="xt", name="xt")
        for _t in range(T):
            nc.sync.dma_start(xt_sb[:, _t * M:(_t + 1) * M],
                              xt_d[:, _t * M:(_t + 1) * M])
        whzr0_sb = load("whzr0", whzr0_d, (128, 1024))
        whh0_sb = load("whh0", whh0_d, (128, 512))
        wx1_sb = load("wx1", wx1_d, (128, 1536))
        whzr1_sb = load("whzr1", whzr1_d, (128, 1024))
        whh1_sb = load("whh1", whh1_d, (128, 512))
        bias_sb = load("bias", bias_d, (128, 12), DT.float32)

        hst = {}
        for L in (0, 1):
            for ci, (m0, mw) in enumerate(CHUNKS):
                for pp in (0, 1):
                    nm = f"h{L}_{ci}_{pp}"
                    hst[(L, ci, pp)] = hpool.tile([128, 2 * mw], bf, tag=nm, name=nm)

        NCH = len(CHUNKS)

        def make_xp0(t, ci):
            m0, mw = CHUNKS[ci]
            x_rhs = xt_sb[:, t * M + m0: t * M + m0 + mw]

            def xp0(g, out_ap, more):
                nc.tensor.matmul(out_ap, wx0_sb[:, g * 128:(g + 1) * 128],
                                 x_rhs, start=True, stop=not more)
            return xp0

        def make_xp1(t, ci):
            mw = CHUNKS[ci][1]
            h0_new = hst[(0, ci, t % 2)]

            def xp1(g, out_ap, more):
                for k in (0, 1):
                    nc.tensor.matmul(
                        out_ap, wx1_sb[:, k * 768 + g * 128: k * 768 + (g + 1) * 128],
                        h0_new[:, k * mw:(k + 1) * mw],
                        start=(k == 0), stop=(k == 1) and not more)
            return xp1

        for t in range(T):
            pp_w = t % 2
            pp_r = 1 - pp_w
            for L, make_xp, whzr_sb, whh_sb, bcol0 in (
                    (0, make_xp0, whzr0_sb, whh0_sb, None),
                    (1, make_xp1, whzr1_sb, whh1_sb, 6)):
                # Software-pipelined emission: the candidate stage of chunk
                # ci-SKEW is emitted after the z/r stage of chunk ci, so the
                # PE instruction stream never stalls on the sigmoid -> r*h
                # chain of the chunk it just fed, and the scalar engine gets
                # enough lead time to vacate PSUM banks before the PE's next
                # group-start reclaims them.
                SKEW = 1
                stage_a = {}
                for ci in range(NCH + SKEW):
                    if ci < NCH:
                        mw = CHUNKS[ci][1]
                        uid = f"L{L}t{t}c{ci}"
                        stage_a[ci] = _emit_zr_stage(
                            nc, psum, work, t, mw, make_xp(t, ci), whzr_sb,
                            bias_sb, bcol0, hst[(L, ci, pp_r)], uid)
                    if ci >= SKEW:
                        cj = ci - SKEW
                        mw = CHUNKS[cj][1]
                        uid = f"L{L}t{t}c{cj}"
                        s_z, rh = stage_a.pop(cj)
                        _emit_cand_stage(
                            nc, psum, work, t, mw, make_xp(t, cj), whh_sb,
                            bias_sb, bcol0, hst[(L, cj, pp_r)],
                            hst[(L, cj, pp_w)], s_z, rh, uid)

        ppf = (T - 1) % 2
        for L in (0, 1):
            for ci, (m0, mw) in enumerate(CHUNKS):
                nc.sync.dma_start(out_d[L, ci, :, 0:2 * mw], hst[(L, ci, ppf)][:])

    nc.compile()
    return nc


def _prep_weights(inputs):
    def bf(x):
        return np.ascontiguousarray(np.asarray(x, np.float32), dtype=BF16)

    def kstack(w):  # (256, C) -> (128, 2*C) with [K0 | K1] on cols
        w = np.asarray(w, np.float32)
        return bf(np.concatenate([w[:128], w[128:]], axis=1))

    bias = np.zeros((128, 12), np.float32)
    ball = {}
    for L, (bx, bhzr, bhh) in enumerate(
            [(inputs["bx0"], inputs["bhzr0"], inputs["bhh0"]),
             (inputs["bx1"], inputs["bhzr1"], inputs["bhh1"])]):
        bz = bx[:H] + bhzr[:H]
        br = bx[H:2 * H] + bhzr[H:2 * H]
        bc = bx[2 * H:] + bhh
        ball[L] = np.concatenate([bz, br, bc])
        for gi, v in enumerate((bz, br, bc)):
            bias[:, L * 6 + 2 * gi] = v[:128]
            bias[:, L * 6 + 2 * gi + 1] = v[128:]

    # layer 0 biases ride the x-projection matmul as a third lhsT row
    # (the matching rhs row is all-ones)
    wx0 = np.concatenate([np.asarray(inputs["Wx0"], np.float32),
                          ball[0][None, :]], axis=0)
    return {
        "wx0": bf(wx0),
        "whzr0": kstack(inputs["Whzr0"]),
        "whh0": kstack(inputs["Whh0"]),
        "wx1": kstack(inputs["Wx1"]),
        "whzr1": kstack(inputs["Whzr1"]),
        "whh1": kstack(inputs["Whh1"]),
        "bias": bias,
    }


def kernel(**inputs):
    X = np.asarray(inputs["X"], np.float32)
    shared = _prep_weights(inputs)

    if "nc" not in _CACHE:
        _CACHE["nc"] = _build_nc()
    nc = _CACHE["nc"]

    in_maps = []
    ones = np.ones((1, T * M), np.float32)
    for c in range(NCORES):
        Xc = X[c * B_SH:(c + 1) * B_SH]                      # (8, T, N, D)
        xt = np.ascontiguousarray(Xc.transpose(3, 1, 0, 2)).reshape(D, T * M)
        m = dict(shared)
        m["xt"] = np.ascontiguousarray(np.concatenate([xt, ones], axis=0),
                                       dtype=BF16)
        in_maps.append(m)
    _CACHE["in_maps"] = in_maps

    res = bass_utils.run_bass_kernel_spmd(nc, in_maps, core_ids=list(range(NCORES)))

    out = np.empty((2, B, N, H), np.float32)
    for c in range(NCORES):
        arr = np.asarray(res.results[c]["out"], dtype=np.float32)  # (2,6,128,2*CWMAX)
        per_core = np.empty((2, M, H), np.float32)
        for ci, (m0, mw) in enumerate(CHUNKS):
            blk = arr[:, ci, :, :2 * mw].reshape(2, 128, 2, mw)
            # [l, p, k, j] -> feature k*128+p, row m0+j
            per_core[:, m0:m0 + mw, :] = blk.transpose(0, 3, 2, 1).reshape(2, mw, H)
        out[:, c * B_SH:(c + 1) * B_SH] = per_core.reshape(2, B_SH, N, H)
    return out



# revision 2
# speedup vs baseline: 1.0207x; 1.0207x over previous
"""Two-layer GRU encoder (B=64, T=12, N=325, D=2, H=256) on 8 TRN2 NeuronCores.

v2.5: fp16 compute + fp8 (e4m3) DoubleRow matmuls on the z/r paths. Hidden
states live in fp16 tiles; packed e4m3 copies for the DoubleRow moving
operands are produced by DVE casts each step (h0's fp8 copy is shared by the
layer-0 recurrence and the layer-1 x-projection). Activations are fused
across PSUM banks (sigmoid over 4 banks, tanh over 2); layer-1 biases ride
K=1 ones-row matmuls so no activation needs a bias operand.

The whole run is ONE flat software pipeline over (t, layer, chunk) stages
with SKEW=3 between the z/r stage and the candidate stage. PSUM is managed
manually inside a single 8-bank tile: z/r stages alternate between two
4-bank regions so stage i+1 never waits on sigma(i); each candidate borrows
the z-banks of its own stage's region (free between sigma and the region's
next reuse). Subtile dependency tracking provides the hazards.
"""

import numpy as np
import ml_dtypes
from contextlib import ExitStack

import concourse.bass as bass
import concourse.tile as tile
from concourse import bacc, mybir
from concourse import bass_utils

F16 = np.float16
E4M3 = ml_dtypes.float8_e4m3fn
AF = mybir.ActivationFunctionType
DT = mybir.dt
DR = mybir.MatmulPerfMode.DoubleRow

H = 256
T = 12
B = 64
N = 325
D = 2
NCORES = 8
B_SH = B // NCORES            # 8
M = B_SH * N                  # 2600
_CWS = [434, 434, 434, 434, 432, 432]
CHUNKS = []
_o = 0
for _w in _CWS:
    CHUNKS.append((_o, _w))
    _o += _w
NCH = len(CHUNKS)
PW = 448                      # padded half stride in fp16 h tiles
SKEW = 3

_CACHE = {}


def _build_nc():
    nc = bacc.Bacc("TRN2", target_bir_lowering=False, debug=False,
                   enable_asserts=False)
    f16 = DT.float16
    f8 = DT.float8e4
    f32 = DT.float32

    xt_d = nc.dram_tensor("xt", (3, T * M), f16, kind="ExternalInput").ap()
    wx0_d = nc.dram_tensor("wx0", (3, 768), f16, kind="ExternalInput").ap()
    whzr0_d = nc.dram_tensor("whzr0", (128, 1024), f8, kind="ExternalInput").ap()
    whh0_d = nc.dram_tensor("whh0", (128, 512), f16, kind="ExternalInput").ap()
    wx1zr_d = nc.dram_tensor("wx1zr", (128, 1024), f8, kind="ExternalInput").ap()
    wx1c_d = nc.dram_tensor("wx1c", (128, 512), f16, kind="ExternalInput").ap()
    whzr1_d = nc.dram_tensor("whzr1", (128, 1024), f8, kind="ExternalInput").ap()
    whh1_d = nc.dram_tensor("whh1", (128, 512), f16, kind="ExternalInput").ap()
    bias1_d = nc.dram_tensor("bias1", (1, 768), f16, kind="ExternalInput").ap()
    out_d = nc.dram_tensor("out", (2, NCH, 128, 2 * PW), f16,
                           kind="ExternalOutput").ap()

    with tile.TileContext(nc) as tc, ExitStack() as ctx:
        const = ctx.enter_context(tc.tile_pool(name="const", bufs=1))
        hpool = ctx.enter_context(tc.tile_pool(name="hstate", bufs=1))
        work = ctx.enter_context(tc.tile_pool(name="work", bufs=4))
        psum = ctx.enter_context(tc.tile_pool(name="psum", bufs=1, space="PSUM"))

        def load(name, dram, shape, dtype):
            t_ = const.tile(list(shape), dtype, tag=name, name=name)
            nc.sync.dma_start(t_[:], dram[:])
            return t_

        wx0 = load("wx0", wx0_d, (3, 768), f16)
        xt = const.tile([3, T * M], f16, tag="xt", name="xt")
        for _t in range(T):
            nc.sync.dma_start(xt[:, _t * M:(_t + 1) * M],
                              xt_d[:, _t * M:(_t + 1) * M])
        whzr0 = load("whzr0", whzr0_d, (128, 1024), f8)
        whh0 = load("whh0", whh0_d, (128, 512), f16)
        wx1zr = load("wx1zr", wx1zr_d, (128, 1024), f8)
        wx1c = load("wx1c", wx1c_d, (128, 512), f16)
        whzr1 = load("whzr1", whzr1_d, (128, 1024), f8)
        whh1 = load("whh1", whh1_d, (128, 512), f16)
        bias1 = load("bias1", bias1_d, (1, 768), f16)
        ones = const.tile([1, 512], f16, tag="ones", name="ones")
        nc.vector.memset(ones[:], 1.0)

        # single 8-bank PSUM tile, manually banked
        pp8 = psum.tile([128, 8, 512], f32, tag="pp8", name="pp8", bufs=1)

        # PE warmup: ramp DVFS before the pipeline starts (banks 4-5)
        wuscr = work.tile([128, 512], f16, tag="wus", name="wuscr", bufs=1)
        for r in range(5):
            for g in range(2):
                nc.tensor.matmul(pp8[:, 4 + g, :], wx1c[:, 0:128],
                                 wx1c[:, 0:512], start=True, stop=True)
        nc.scalar.copy(wuscr[:], pp8[:, 4, :])
        nc.scalar.copy(wuscr[:], pp8[:, 5, :])

        # fp16 hidden states, halves at [0:mw] and [PW:PW+mw]
        hst = {}
        h8st = {}
        for L in (0, 1):
            for ci in range(NCH):
                for pp in (0, 1):
                    nm = f"h{L}_{ci}_{pp}"
                    hst[(L, ci, pp)] = hpool.tile([128, 2 * PW], f16,
                                                  tag=nm, name=nm)
                    nm8 = f"h8_{L}_{ci}_{pp}"
                    h8st[(L, ci, pp)] = hpool.tile([128, 1024], f8,
                                                   tag=nm8, name=nm8)

        def h_f16(tile_, mw):  # [128, 2, mw] fp16 view (halves at stride PW)
            return tile_[:, :].rearrange("p (k m) -> p k m", k=2)[:, :, 0:mw]

        def h8_v(tile_, mw):  # [128, 2, mw] packed e4m3 view (stride 512)
            return tile_[:, :].rearrange("p (k m) -> p k m", k=2)[:, :, 0:mw]

        def wdr(w, g):  # [128, 2, 128] DR weight view for gate-half g
            return w[:, g * 256:(g + 1) * 256].rearrange("p (k f) -> p k f", k=2)

        # weight gate order: cols [z | r]; banks in region: [za zb ra rb]
        def emit_zr(nc_, t, L, ci, reg, s_zr):
            m0, mw = CHUNKS[ci]
            first = t == 0
            pp_r = 1 - t % 2
            # r-gate banks (reg+2, reg+3) first: they only wait sigma(i-2);
            # z-banks (reg+0, reg+1) last: they also wait the tanh of the
            # cand stage that borrowed them one slot ago.
            border = (2, 3, 0, 1)

            def dr_block(w, h8t, start):
                for g in border:
                    nc_.tensor.matmul(pp8[:, reg + g, 0:mw], wdr(w, g),
                                      h8_v(h8t, mw), start=start, stop=False,
                                      perf_mode=DR)

            if L == 0:
                if not first:
                    dr_block(whzr0, h8st[(0, ci, pp_r)], True)
                x_rhs = xt[:, t * M + m0: t * M + m0 + mw]
                for g in border:
                    nc_.tensor.matmul(pp8[:, reg + g, 0:mw],
                                      wx0[:, g * 128:(g + 1) * 128],
                                      x_rhs, start=first, stop=True)
            else:
                h08n = h8st[(0, ci, t % 2)]
                if not first:
                    dr_block(whzr1, h8st[(1, ci, pp_r)], True)
                dr_block(wx1zr, h08n, first)
                for g in border:
                    nc_.tensor.matmul(pp8[:, reg + g, 0:mw],
                                      bias1[:, g * 128:(g + 1) * 128],
                                      ones[:, 0:mw], start=False, stop=True)
            # fused sigmoid over the region's 4 banks -> s_zr [za zb ra rb]
            nc_.scalar.activation(
                s_zr[:, :].rearrange("p (g m) -> p g m", g=4)[:, :, 0:mw],
                pp8[:, reg:reg + 4, 0:mw], AF.Sigmoid)

        def emit_cand(nc_, t, L, ci, reg, s_zr, c):
            # c banks = z-banks (reg+0, reg+1) of this stage's own region
            m0, mw = CHUNKS[ci]
            first = t == 0
            pp_r = 1 - t % 2
            pp_w = t % 2
            hp = hst[(L, ci, pp_r)]
            hn = hst[(L, ci, pp_w)]
            rh = None
            if not first:
                rh = work.tile([128, 2 * PW], DT.float16, tag="rh",
                               name=f"rh{L}{ci}")
                s_r = s_zr[:, 2 * PW:].rearrange("p (k m) -> p k m", k=2)[:, :, 0:mw]
                nc_.vector.tensor_mul(h_f16(rh, mw), s_r, h_f16(hp, mw))
            whh = whh0 if L == 0 else whh1
            if L == 0:
                x_rhs = xt[:, t * M + m0: t * M + m0 + mw]
                for g in range(2):
                    nc_.tensor.matmul(pp8[:, reg + g, 0:mw],
                                      wx0[:, 512 + g * 128: 512 + (g + 1) * 128],
                                      x_rhs, start=True, stop=first)
            else:
                h0n = hst[(0, ci, pp_w)]
                for g in range(2):
                    for k in range(2):
                        nc_.tensor.matmul(
                            pp8[:, reg + g, 0:mw],
                            wx1c[:, k * 256 + g * 128: k * 256 + (g + 1) * 128],
                            h0n[:, k * PW:k * PW + mw],
                            start=(k == 0), stop=False)
                for g in range(2):
                    nc_.tensor.matmul(pp8[:, reg + g, 0:mw],
                                      bias1[:, 512 + g * 128: 512 + (g + 1) * 128],
                                      ones[:, 0:mw], start=False, stop=first)
            if not first:
                for g in range(2):
                    for k in range(2):
                        nc_.tensor.matmul(
                            pp8[:, reg + g, 0:mw],
                            whh[:, k * 256 + g * 128: k * 256 + (g + 1) * 128],
                            rh[:, k * PW:k * PW + mw],
                            start=False, stop=(k == 1))
            # fused tanh over the 2 borrowed banks
            nc_.scalar.activation(
                c[:, :].rearrange("p (g m) -> p g m", g=2)[:, :, 0:mw],
                pp8[:, reg:reg + 2, 0:mw], AF.Tanh)
            # blend
            s_z = s_zr[:, 0:2 * PW].rearrange("p (k m) -> p k m", k=2)[:, :, 0:mw]
            cv = c[:, :].rearrange("p (k m) -> p k m", k=2)[:, :, 0:mw]
            if first:
                nc_.vector.tensor_mul(h_f16(hn, mw), s_z, cv)
            else:
                d = work.tile([128, 2 * PW], DT.float16, tag="d", name=f"d{L}{ci}")
                nc_.vector.tensor_sub(h_f16(d, mw), cv, h_f16(hp, mw))
                zd = work.tile([128, 2 * PW], DT.float16, tag="zd", name=f"zd{L}{ci}")
                nc_.vector.tensor_mul(h_f16(zd, mw), s_z, h_f16(d, mw))
                nc_.vector.tensor_add(h_f16(hn, mw), h_f16(hp, mw), h_f16(zd, mw))
            # packed e4m3 copy for next-step DR reads (and L1 xp for L==0);
            # dead at the last step for L==1
            if L == 0 or t < T - 1:
                h8n = h8st[(L, ci, pp_w)]
                nc_.vector.tensor_copy(h8n[:, 0:mw], hn[:, 0:mw])
                nc_.vector.tensor_copy(h8n[:, 512:512 + mw], hn[:, PW:PW + mw])

        stages = [(t, L, ci) for t in range(T) for L in (0, 1)
                  for ci in range(NCH)]
        pending = {}
        for si in range(len(stages) + SKEW):
            if si < len(stages):
                t, L, ci = stages[si]
                s_zr = work.tile([128, 4 * PW], DT.float16, tag="szr",
                                 name=f"szr{L}{t}{ci}", bufs=SKEW + 2)
                emit_zr(nc, t, L, ci, 4 * (si % 2), s_zr)
                pending[si] = s_zr
            if si >= SKEW:
                sj = si - SKEW
                t, L, cj = stages[sj]
                c = work.tile([128, 2 * PW], DT.float16, tag="c",
                              name=f"c{L}{t}{cj}")
                emit_cand(nc, t, L, cj, 4 * (sj % 2), pending.pop(sj), c)

        ppf = (T - 1) % 2
        for L in (0, 1):
            for ci in range(NCH):
                nc.sync.dma_start(out_d[L, ci], hst[(L, ci, ppf)][:])

    nc.compile()
    return nc


def _prep_weights(inputs):
    def f32(x):
        return np.asarray(x, np.float32)

    def q8c(x):
        return np.clip(f32(x), -240, 240).astype(E4M3)

    def dr_pack(W):  # (256, G*128) -> (128, G*256) DR layout
        G = W.shape[1] // 128
        out = np.zeros((128, G * 256), np.float32)
        for g in range(G):
            for k in range(2):
                out[:, g * 256 + k * 128:g * 256 + (k + 1) * 128] = \
                    W[k * 128:(k + 1) * 128, g * 128:(g + 1) * 128]
        return out

    def kstack(W):  # (256, C) -> (128, 2C)
        return np.concatenate([W[:128], W[128:]], axis=1)

    ball = {}
    for L in (0, 1):
        bx = f32(inputs[f"bx{L}"])
        bhzr = f32(inputs[f"bhzr{L}"])
        bhh = f32(inputs[f"bhh{L}"])
        ball[L] = np.concatenate([bx[:2 * H] + bhzr, bx[2 * H:] + bhh])

    wx0 = np.concatenate([f32(inputs["Wx0"]), ball[0][None, :]], axis=0)
    wx1 = f32(inputs["Wx1"])
    return {
        "wx0": wx0.astype(F16),
        "whzr0": q8c(dr_pack(f32(inputs["Whzr0"]))),
        "whh0": kstack(f32(inputs["Whh0"])).astype(F16),
        "wx1zr": q8c(dr_pack(wx1[:, :2 * H])),
        "wx1c": kstack(wx1[:, 2 * H:]).astype(F16),
        "whzr1": q8c(dr_pack(f32(inputs["Whzr1"]))),
        "whh1": kstack(f32(inputs["Whh1"])).astype(F16),
        "bias1": ball[1][None, :].astype(F16),
    }


def kernel(**inputs):
    X = np.asarray(inputs["X"], np.float32)
    shared = _prep_weights(inputs)

    if "nc" not in _CACHE:
        _CACHE["nc"] = _build_nc()
    nc = _CACHE["nc"]

    in_maps = []
    ones = np.ones((1, T * M), np.float32)
    for c in range(NCORES):
        Xc = X[c * B_SH:(c + 1) * B_SH]                      # (8, T, N, D)
        xt = np.ascontiguousarray(Xc.transpose(3, 1, 0, 2)).reshape(D, T * M)
        m = dict(shared)
        m["xt"] = np.concatenate([xt, ones], axis=0).astype(F16)
        in_maps.append(m)
    _CACHE["in_maps"] = in_maps

    res = bass_utils.run_bass_kernel_spmd(nc, in_maps, core_ids=list(range(NCORES)))

    out = np.empty((2, B, N, H), np.float32)
    for c in range(NCORES):
        arr = np.asarray(res.results[c]["out"], dtype=np.float32)  # (2,6,128,2PW)
        per_core = np.empty((2, M, H), np.float32)
        for ci, (m0, mw) in enumerate(CHUNKS):
            blk = np.stack([arr[:, ci, :, 0:mw], arr[:, ci, :, PW:PW + mw]], axis=2)
            per_core[:, m0:m0 + mw, :] = blk.transpose(0, 3, 2, 1).reshape(2, mw, H)
        out[:, c * B_SH:(c + 1) * B_SH] = per_core.reshape(2, B_SH, N, H)
    return out
